# revision 1
# baseline (speedup 1.0000x reference)
"""Talking-heads attention TRN2 kernel v3 — bf16, PE transposes, lean evictions.

Structure per core (8 batches, weights replicated):
  phase 0: weights f32r PE-transposed (4-chained psum groups), evict-cast to
           bf16 (q rows pre-scaled); w_l/w_w -> block-diag bd1/bd2; RPE bias
           gathered (unmixed) into packed rows [(nb,h), (j, m)], bf16.
  phase A (per 4-batch group): x loaded (junk rows filled), f32r PE-transpose
           -> xT bf16; QKV GEMM weight-stationary across the group.
  phase B (per batch): QK^T per head -> contiguous evict to sqT [m, h, n250];
           pack-transpose with strided (nb,h)-gather input AP -> pkT psum bf16
           (4-chained groups); DMA pkT -> pkS sbuf; gpsimd bias-add -> pkb;
           premix (bd1 matmul); exp from psum (no max-sub: logits tiny);
           DVE sum+recip; normalizer folded into bd2 rows (gpsimd); postmix
           fused with transpose-back (lhsT=et), 4-chained psat groups;
           cast-evict -> atw; AV per head with b_w*colsum(v) added via
           per-partition bias at psav eviction; projection with b_proj as an
           extra contraction row; psum -> DRAM DMA.

b_l is softmax-invariant and dropped.  n = nb*25 + j (nb<10, j<25).
"""
import numpy as np
from contextlib import ExitStack

import concourse.bass as bass
import concourse.tile as tile
from concourse import bacc, mybir, library_config
from concourse.bass_utils import run_bass_kernel_spmd

F32 = mybir.dt.float32
F32R = mybir.dt.float32r
BF16 = mybir.dt.bfloat16
I32 = mybir.dt.int32
I16 = mybir.dt.int16
AX = mybir.AxisListType.X
EXP = mybir.ActivationFunctionType.Exp
ADD = mybir.AluOpType.add
MULT = mybir.AluOpType.mult

NCORES = 8
B, N, C, H, D = 64, 245, 768, 12, 64
BLOC = B // NCORES
E = 3 * C
NBKT = 1698
SCALE = D ** -0.5
NGRP = 10                   # nb groups; packed rows = NGRP*H = 120
NJ = 25                     # n = nb*NJ + j
PR = NGRP * H               # 120
NN = NGRP * NJ              # 250 (n padded; q cols 245..249 junk)
NP = 256                    # padded m stride in xT/qkT
GSZ = 4
CC = C // 128               # 6
MCS = [(0, 128), (128, 117)]
NI = NJ * N                 # 6125
NIP = 6128


def _emit(ctx: ExitStack, tc, io):
    nc = tc.nc
    x_d, wqkv_d, wproj_d, bproj_d, wl_d, ww_d, bw_d, rpe_d, rel_d, out_d = io

    const = ctx.enter_context(tc.tile_pool(name="const", bufs=1))
    ctx0 = ExitStack()
    tmp = ctx0.enter_context(tc.tile_pool(name="tmp", bufs=1))
    ps = ctx.enter_context(tc.tile_pool(name="ps", bufs=1, space="PSUM"))

    identB = const.tile([128, 128], BF16)
    from concourse.masks import make_identity
    make_identity(nc, identB[:])
    identF = const.tile([128, 128], F32)
    make_identity(nc, identF[:])

    # ---- small weights ----
    wl_sb = tmp.tile([12, 12], F32, tag="wsml")
    nc.sync.dma_start(out=wl_sb[:], in_=wl_d[:, :])
    ps12 = ps.tile([12, 12], F32, tag="a")
    nc.tensor.transpose(out=ps12[:], in_=wl_sb[:], identity=identF[:12, :12])
    wlT = tmp.tile([12, 12], BF16, tag="wsml2")
    nc.scalar.copy(out=wlT[:], in_=ps12[:])

    ww_sb = tmp.tile([12, 12], F32, tag="wsml")
    nc.sync.dma_start(out=ww_sb[:], in_=ww_d[:, :])
    ps12b = ps.tile([12, 12], F32, tag="b")
    nc.tensor.transpose(out=ps12b[:], in_=ww_sb[:], identity=identF[:12, :12])
    wwT = tmp.tile([12, 12], BF16, tag="wsml2b")
    nc.scalar.copy(out=wwT[:], in_=ps12b[:])

    bd1 = const.tile([128, PR], BF16)   # [(nb,h), (nb,g)] = wl[g,h]
    nc.vector.memset(bd1[:], 0.0)
    bd2 = const.tile([128, PR], BF16)   # [(nb,h), (nb,g)] = ww[g,h]
    nc.vector.memset(bd2[:], 0.0)
    for nb in range(NGRP):
        s = nb * H
        nc.gpsimd.dma_start(out=bd1[s:s + H, s:s + H], in_=wlT[:])
        nc.gpsimd.dma_start(out=bd2[s:s + H, s:s + H], in_=wwT[:])

    # b_w broadcast: bw_cols[p, h] = b_w[h] on 64 partitions
    bw_cols = const.tile([64, H], F32)
    for h in range(H):
        nc.gpsimd.dma_start(
            out=bw_cols[:, h:h + 1],
            in_=bw_d[h:h + 1].unsqueeze(0).to_broadcast([64, 1]))
    # b_proj as an extra (row-0-only) contraction step for the projection
    bproj_row = const.tile([1, C], F32)
    nc.sync.dma_start(out=bproj_row[:], in_=bproj_d[:].unsqueeze(0))
    e0_col = const.tile([128, 128], BF16)   # only row 0 is ones
    nc.vector.memset(e0_col[:], 0.0)
    nc.gpsimd.memset(e0_col[0:1, :], 1.0)
    bprow128 = const.tile([128, C], BF16)   # only row 0 is b_proj
    nc.vector.memset(bprow128[:], 0.0)
    nc.gpsimd.tensor_copy(out=bprow128[0:1, :], in_=bproj_row[:])
    ones_col = const.tile([128, 1], BF16)
    nc.vector.memset(ones_col[:], 1.0)

    # ---- big weights: load f32, PE-transpose (f32r), evict-cast to bf16 ----
    wqkvT = const.tile([128, CC, E], BF16)     # [c, cc, e]
    wprojT = const.tile([128, CC, C], BF16)
    for w_i, (w_d, wT, necs, scl) in enumerate((
            (wqkv_d, wqkvT, E // 128, True), (wproj_d, wprojT, CC, False))):
        for ec in range(necs):
            wt = tmp.tile([128, C], F32, tag="wload", bufs=2)
            nc.sync.dma_start(out=wt[:], in_=w_d[ec * 128:(ec + 1) * 128, :])
            wb = tmp.tile([128, C], BF16, tag="wcast", bufs=2)
            if scl and ec < 6:
                nc.vector.tensor_scalar_mul(wb[:], wt[:], SCALE)
            else:
                nc.scalar.copy(out=wb[:], in_=wt[:])
            for cc2 in range(3):   # 2 cc per psum tile
                psw = ps.tile([128, 2, 128], BF16, tag=("e", "f")[cc2 % 2],
                              name="psw")
                for k in range(2):
                    cc = cc2 * 2 + k
                    nc.tensor.matmul(
                        out=psw[:, k, :], lhsT=wb[:, cc * 128:(cc + 1) * 128],
                        rhs=identB[:, :], is_transpose=True,
                        start=(k == 0), stop=(k == 1))
                dst = wT[:, cc2 * 2:cc2 * 2 + 2, ec * 128:(ec + 1) * 128]
                if (ec + cc2) % 2 == 0:
                    nc.scalar.copy(out=dst, in_=psw[:])
                else:
                    nc.vector.tensor_copy(out=dst, in_=psw[:])

    # ---- RPE bias gather into packed rows [(nb,h), (j, m)] ----
    rel_flat = rel_d.rearrange("n m -> (n m)")
    trep1 = tmp.tile([128, NBKT], F32, tag="trep1")
    nc.vector.memset(trep1[:], 0.0)
    trep2 = tmp.tile([32, NBKT], F32, tag="trep2")
    nc.vector.memset(trep2[:], 0.0)
    for h in range(H):
        for nb in range(8):
            nc.sync.dma_start(
                out=trep1[nb * 16 + h:nb * 16 + h + 1, :], in_=rpe_d[h:h + 1, :])
        for nb in range(2):
            nc.sync.dma_start(
                out=trep2[nb * 16 + h:nb * 16 + h + 1, :], in_=rpe_d[h:h + 1, :])

    idx32a = tmp.tile([128, NIP // 16], I32, tag="idx32a")
    for nb in range(8):
        a = nb * NI
        nc.sync.dma_start(
            out=idx32a[nb * 16:(nb + 1) * 16, :],
            in_=rel_flat[a:a + NIP].rearrange("(s p) -> p s", p=16))
    idx32b = tmp.tile([32, NIP // 16], I32, tag="idx32b")
    nc.vector.memset(idx32b[:], 0)
    nc.sync.dma_start(
        out=idx32b[0:16, :],
        in_=rel_flat[8 * NI:8 * NI + NIP].rearrange("(s p) -> p s", p=16))
    nc.sync.dma_start(
        out=idx32b[16:32, 0:306],
        in_=rel_flat[9 * NI:9 * NI + 4896].rearrange("(s p) -> p s", p=16))
    nc.sync.dma_start(
        out=idx32b[16:20, 306:307],
        in_=rel_flat[9 * NI + 4896:9 * NI + 4900].rearrange("(s p) -> p s", p=4))
    idx16a = tmp.tile([128, NIP // 16], I16, tag="idx16a")
    nc.vector.tensor_copy(out=idx16a[:], in_=idx32a[:])
    idx16b = tmp.tile([32, NIP // 16], I16, tag="idx16b")
    nc.vector.tensor_copy(out=idx16b[:], in_=idx32b[:])

    nc.gpsimd.load_library(library_config.ap_gather)
    bias_g1 = tmp.tile([128, NIP], F32, tag="bg1")
    nc.gpsimd.ap_gather(
        out_ap=bias_g1[:], in_ap=trep1[:].unsqueeze(2), idxs_ap=idx16a[:],
        channels=128, num_elems=NBKT, d=1, num_idxs=NIP)
    bias_g2 = tmp.tile([32, NIP], F32, tag="bg2")
    nc.gpsimd.ap_gather(
        out_ap=bias_g2[:], in_ap=trep2[:].unsqueeze(2), idxs_ap=idx16b[:],
        channels=32, num_elems=NBKT, d=1, num_idxs=NIP)
    nc.gpsimd.load_library(library_config.standard)

    bias_c1 = tmp.tile([128, NIP], BF16, tag="bc1")
    nc.vector.tensor_copy(out=bias_c1[:], in_=bias_g1[:])
    bias_c2 = tmp.tile([32, NIP], BF16, tag="bc2")
    nc.vector.tensor_copy(out=bias_c2[:], in_=bias_g2[:])

    bias_pk = const.tile([PR, NJ, N], BF16)
    for nb in range(NGRP):
        src = bias_c1 if nb < 8 else bias_c2
        so = (nb if nb < 8 else nb - 8) * 16
        nc.gpsimd.dma_start(
            out=bias_pk[nb * H:(nb + 1) * H, :, :],
            in_=src[so:so + H, 0:NI].rearrange("p (j m) -> p j m", j=NJ))

    ctx0.close()

    # ---- streaming pools ----
    xb_p = ctx.enter_context(tc.tile_pool(name="xb", bufs=2))
    xT_p = ctx.enter_context(tc.tile_pool(name="xT", bufs=1))
    qkT_p = ctx.enter_context(tc.tile_pool(name="qkT", bufs=1))
    v_p = ctx.enter_context(tc.tile_pool(name="v", bufs=1))
    sq_p = ctx.enter_context(tc.tile_pool(name="sq", bufs=2))
    pk_p = ctx.enter_context(tc.tile_pool(name="pk", bufs=3))
    et_p = ctx.enter_context(tc.tile_pool(name="et", bufs=3))
    sm_p = ctx.enter_context(tc.tile_pool(name="sm", bufs=4))
    atw_p = ctx.enter_context(tc.tile_pool(name="atw", bufs=2))
    oT_p = ctx.enter_context(tc.tile_pool(name="oT", bufs=2))

    ev = [
        lambda out, in_: nc.scalar.copy(out=out, in_=in_),
        lambda out, in_: nc.vector.tensor_copy(out=out, in_=in_),
    ]

    for gg in range(BLOC // GSZ):
        # ---- phase A ----
        xT_all = xT_p.tile([128, CC, GSZ * NP], BF16, tag="xT")
        qkT = qkT_p.tile([128, 12, GSZ * NP], BF16, tag="qkT")
        v_all = v_p.tile([128, 2, GSZ, C], BF16, tag="v")
        for bs in range(GSZ):
            b = gg * GSZ + bs
            xb = xb_p.tile([128, 2, C], F32, tag="xb")
            for mc, (mo, msz) in enumerate(MCS):
                nc.sync.dma_start(out=xb[:msz, mc, :], in_=x_d[b, mo:mo + msz, :])
            nc.sync.dma_start(out=xb[117:128, 1, :], in_=x_d[b, 234:245, :])
            xbb = xb_p.tile([128, 2, C], BF16, tag="xbb")
            nc.scalar.copy(out=xbb[:, 0, :], in_=xb[:, 0, :])
            nc.scalar.copy(out=xbb[:, 1, :], in_=xb[:, 1, :])
            # x^T via bf16 PE transposes, 4 per psum tile, evicted to xT
            for c4 in range(3):   # (cc pair) x (mc pair) per tile
                psx = ps.tile([128, 2, 2, 128], BF16, tag=("e", "f")[c4 % 2],
                              name="psx")
                for k in range(2):
                    cc = c4 * 2 + k
                    for mc in range(2):
                        nc.tensor.matmul(
                            out=psx[:, k, mc, :],
                            lhsT=xbb[:, mc, cc * 128:(cc + 1) * 128],
                            rhs=identB[:, :], is_transpose=True,
                            start=(k == 0 and mc == 0), stop=(k == 1 and mc == 1))
                ev[(bs + c4) % 2](
                    xT_all[:, c4 * 2:c4 * 2 + 2, bs * NP:(bs + 1) * NP].rearrange(
                        "p c (q x) -> p c q x", q=2),
                    psx[:])
        # QKV (q scaled in weights): qkT [hd, n]
        for ec in range(12):
            pst = [ps.tile([128, NP], F32, tag=tg, name=f"psq{tg}")
                   for tg in ("a", "b", "c", "d")]
            for cc in range(CC):
                for bs in range(GSZ):
                    nc.tensor.matmul(
                        out=pst[bs][:, :],
                        lhsT=wqkvT[:, cc, ec * 128:(ec + 1) * 128],
                        rhs=xT_all[:, cc, bs * NP:(bs + 1) * NP],
                        start=(cc == 0), stop=(cc == CC - 1))
            for bs in range(GSZ):
                ev[(ec * GSZ + bs) % 2](
                    qkT[:, ec, bs * NP:(bs + 1) * NP], pst[bs][:, :])
        # V: [m, hd]
        for bs in range(GSZ):
            for mc, (mo, msz) in enumerate(MCS):
                psv0 = ps.tile([128, 384], F32, tag="c")
                psv1 = ps.tile([128, 384], F32, tag="d")
                psvs = (psv0, psv1)
                for cc in range(CC):
                    for vc in range(2):
                        nc.tensor.matmul(
                            out=psvs[vc][:msz, :],
                            lhsT=xT_all[:, cc, bs * NP + mo:bs * NP + mo + msz],
                            rhs=wqkvT[:, cc, 2 * C + vc * 384:2 * C + (vc + 1) * 384],
                            start=(cc == 0), stop=(cc == CC - 1))
                for vc in range(2):
                    ev[(bs * 2 + mc + vc) % 2](
                        v_all[:msz, mc, bs, vc * 384:(vc + 1) * 384],
                        psvs[vc][:msz, :])

        # ---- phase B ----
        for bs in range(GSZ):
            b = gg * GSZ + bs
            # QK^T per head -> evict into sqT [m, mc, nb, h, j] so that the
            # packed (nb,h) gather at fixed j is a single-stride AP (stride NJ)
            sqT = sq_p.tile([128, 2, NGRP, H, NJ], BF16, tag="sqT")
            for mc in range(2):
                for t in range(6):
                    for h2 in range(2):
                        h = 2 * t + h2
                        psqk = ps.tile([128, NN], F32,
                                       tag=("e", "f")[h % 2], name="psqk")
                        nc.tensor.matmul(
                            out=psqk[:, :],
                            lhsT=qkT[h2 * 64:(h2 + 1) * 64, 6 + t,
                                     bs * NP + mc * 128:bs * NP + (mc + 1) * 128],
                            rhs=qkT[h2 * 64:(h2 + 1) * 64, t, bs * NP:bs * NP + NN],
                            start=True, stop=True)
                        ev[(mc * 12 + h) % 2](
                            sqT[:, mc, :, h, :],
                            psqk[:, :].rearrange("p (nb j) -> p nb j", nb=NGRP))

            # packed attention middle, 2 j per step
            # atw layout [m, mc, g, n]: evictions de-interleave (nb,g);
            # column 250 is ones so AV also emits colsum(v) per head
            atw = atw_p.tile([128, 2, H, NN + 6], BF16, tag="atw")
            nc.vector.memset(atw[:, :, :, NN:NN + 1], 1.0)
            psat = [None, None]
            for jj in range(0, NJ, 2):
                js = min(2, NJ - jj)
                # pack-transpose: strided (nb,h)-gather input, 4-chained group
                pkT = ps.tile([128, 2, 2, 128], BF16, tag=("e", "f")[(jj // 2) % 2],
                              name="pkT")
                for i in range(js):
                    for mc in range(2):
                        src = sqT[:, mc].rearrange(
                            "p nb h j -> p (nb h) j")[:, :, jj + i]
                        nc.tensor.matmul(
                            out=pkT[0:PR, i, mc, :],
                            lhsT=src,
                            rhs=identB[:, :], is_transpose=True,
                            start=(i == 0 and mc == 0),
                            stop=(i == js - 1 and mc == 1))
                pkS = pk_p.tile([PR, 2, 2, 128], BF16, tag="pkS")
                ev[(jj // 2) % 2](pkS[:, 0:js, :, :], pkT[0:PR, 0:js, :, :])
                pkb = pk_p.tile([PR, 2, N], BF16, tag="pkb")
                nc.vector.tensor_tensor(
                    out=pkb[:, 0:js, :],
                    in0=pkS[:, 0:js, :, :].rearrange("p j m c -> p j (m c)")[:, :, 0:N],
                    in1=bias_pk[:, jj:jj + js, :], op=ADD)
                psm = ps.tile([PR, 2, N], F32, tag=("g", "h")[(jj // 2) % 2],
                              name="psm")
                nc.tensor.matmul(out=psm[:, 0:js, :], lhsT=bd1[0:PR, :],
                                 rhs=pkb[:, 0:js, :], start=True, stop=True)
                et = et_p.tile([PR, 2, N], BF16, tag="et")
                nc.scalar.activation(out=et[:, 0:js, :], in_=psm[:, 0:js, :], func=EXP)
                ssum = sm_p.tile([PR, 2], F32, tag="ss")
                nc.vector.tensor_reduce(out=ssum[:, 0:js], in_=et[:, 0:js, :],
                                        axis=AX, op=ADD)
                rec = sm_p.tile([PR, 2], F32, tag="rc")
                nc.vector.reciprocal(out=rec[:, 0:js], in_=ssum[:, 0:js])
                bd2j = sm_p.tile([PR, 2, PR], BF16, tag="b2")
                for i in range(js):
                    nc.vector.tensor_scalar_mul(bd2j[:, i, :], bd2[0:PR, :],
                                                rec[:, i:i + 1])
                for i in range(js):
                    j = jj + i
                    if j % 4 == 0:
                        psat[0] = ps.tile([128, 4, PR], F32, tag="a", name="psat0")
                        psat[1] = ps.tile([128, 4, PR], F32, tag="b", name="psat1")
                    for mc, (mo, msz) in enumerate(MCS):
                        nc.tensor.matmul(
                            out=psat[mc][:msz, j % 4, :],
                            lhsT=et[:, i, mo:mo + msz], rhs=bd2j[:, i, :],
                            start=(j % 4 == 0), stop=(j % 4 == 3 or j == NJ - 1))
                    if j % 4 == 3 or j == NJ - 1:
                        j0 = (j // 4) * 4
                        nj4 = j - j0 + 1
                        for mc, (mo, msz) in enumerate(MCS):
                            dst = atw[:msz, mc, :, 0:NN].rearrange(
                                "p g (nb j) -> p j nb g", nb=NGRP,
                                j=NJ)[:, j0:j0 + nj4]
                            ev[mc](dst,
                                   psat[mc][:msz, 0:nj4, :].rearrange(
                                       "p j (nb g) -> p j nb g", nb=NGRP))

            # AV per head; psav eviction adds b_w*colsum(v) per-partition
            outT = oT_p.tile([128, CC, N], BF16, tag="oT")
            bwcv = sm_p.tile([64, 12], F32, tag="bw")
            for t in range(6):
                psav0 = ps.tile([64, NN + 1], F32, tag="c")
                psav1 = ps.tile([64, NN + 1], F32, tag="d")
                psavs = (psav0, psav1)
                for h2 in range(2):
                    for mc, (mo, msz) in enumerate(MCS):
                        nc.tensor.matmul(
                            out=psavs[h2][:, :],
                            lhsT=v_all[:msz, mc, bs, (2 * t + h2) * 64:(2 * t + h2 + 1) * 64],
                            rhs=atw[:msz, mc, 2 * t + h2, 0:NN + 1],
                            start=(mc == 0), stop=(mc == 1))
                for h2 in range(2):
                    nc.vector.tensor_tensor(
                        out=bwcv[:, 2 * t + h2:2 * t + h2 + 1],
                        in0=psavs[h2][:, NN:NN + 1],
                        in1=bw_cols[:, 2 * t + h2:2 * t + h2 + 1], op=MULT)
                nc.scalar.add(out=outT[0:64, t, :], in_=psav0[:, 0:N],
                              add=bwcv[:, 2 * t:2 * t + 1])
                nc.scalar.add(out=outT[64:128, t, :], in_=psav1[:, 0:N],
                              add=bwcv[:, 2 * t + 1:2 * t + 2])

            # projection; b_proj enters as an extra contraction row
            for mc, (mo, msz) in enumerate(MCS):
                psy0 = ps.tile([128, 384], F32, tag="c")
                psy1 = ps.tile([128, 384], F32, tag="d")
                psyt = (psy0, psy1)
                for cc in range(CC):
                    for half in range(2):
                        nc.tensor.matmul(
                            out=psyt[half][:msz, :],
                            lhsT=outT[:, cc, mo:mo + msz],
                            rhs=wprojT[:, cc, half * 384:(half + 1) * 384],
                            start=(cc == 0), stop=False)
                for half in range(2):
                    nc.tensor.matmul(
                        out=psyt[half][:msz, :],
                        lhsT=e0_col[:, 0:msz],
                        rhs=bprow128[:, half * 384:(half + 1) * 384],
                        start=False, stop=True)
                y = sm_p.tile([128, C], F32, tag="y", bufs=2)
                for half in range(2):
                    ev[half](y[:msz, half * 384:(half + 1) * 384],
                             psyt[half][:msz, :])
                nc.sync.dma_start(out=out_d[b, mo:mo + msz, :], in_=y[:msz, :])


_CACHE = {}


def _build():
    if "nc" in _CACHE:
        return _CACHE["nc"]
    nc = bacc.Bacc("TRN2", target_bir_lowering=False, debug=False, num_devices=NCORES)
    io = (
        nc.dram_tensor("x", [BLOC, N, C], F32, kind="ExternalInput").ap(),
        nc.dram_tensor("w_qkv", [E, C], F32, kind="ExternalInput").ap(),
        nc.dram_tensor("w_proj", [C, C], F32, kind="ExternalInput").ap(),
        nc.dram_tensor("b_proj", [C], F32, kind="ExternalInput").ap(),
        nc.dram_tensor("w_l", [H, H], F32, kind="ExternalInput").ap(),
        nc.dram_tensor("w_w", [H, H], F32, kind="ExternalInput").ap(),
        nc.dram_tensor("b_w", [H], F32, kind="ExternalInput").ap(),
        nc.dram_tensor("rpe_table", [H, NBKT], F32, kind="ExternalInput").ap(),
        nc.dram_tensor("rel_idx", [N, N], I32, kind="ExternalInput").ap(),
        nc.dram_tensor("out", [BLOC, N, C], F32, kind="ExternalOutput").ap(),
    )
    with tile.TileContext(nc) as tc, ExitStack() as ctx:
        _emit(ctx, tc, io)
    nc.compile()
    _CACHE["nc"] = nc
    return nc


def kernel(x, w_qkv, w_proj, b_proj, w_l, b_l, w_w, b_w, rpe_table, rel_idx,
           _trace=False):
    nc = _build()
    shared = {
        "w_qkv": np.ascontiguousarray(w_qkv, np.float32),
        "w_proj": np.ascontiguousarray(w_proj, np.float32),
        "b_proj": np.ascontiguousarray(b_proj, np.float32),
        "w_l": np.ascontiguousarray(w_l, np.float32),
        "w_w": np.ascontiguousarray(w_w, np.float32),
        "b_w": np.ascontiguousarray(b_w, np.float32),
        "rpe_table": np.ascontiguousarray(rpe_table, np.float32),
        "rel_idx": np.ascontiguousarray(rel_idx, np.int32),
    }
    x = np.ascontiguousarray(x, np.float32)
    in_maps = [dict(shared, x=x[i * BLOC:(i + 1) * BLOC]) for i in range(NCORES)]
    res = run_bass_kernel_spmd(nc, in_maps, core_ids=list(range(NCORES)),
                               trace=_trace)
    out = np.concatenate([res.results[i]["out"] for i in range(NCORES)], axis=0)
    if _trace:
        kernel.last_result = res
    return out



# revision 17
# speedup vs baseline: 1.3216x; 1.3216x over previous
"""Talking-heads attention TRN2 kernel v3 — bf16, PE transposes, lean evictions.

Structure per core (8 batches, weights replicated):
  phase 0: weights f32r PE-transposed (4-chained psum groups), evict-cast to
           bf16 (q rows pre-scaled); w_l/w_w -> block-diag bd1/bd2; RPE bias
           gathered (unmixed) into packed rows [(nb,h), (j, m)], bf16.
  phase A (per 4-batch group): x loaded (junk rows filled), f32r PE-transpose
           -> xT bf16; QKV GEMM weight-stationary across the group.
  phase B (per batch): QK^T per head -> contiguous evict to sqT [m, h, n250];
           pack-transpose with strided (nb,h)-gather input AP -> pkT psum bf16
           (4-chained groups); DMA pkT -> pkS sbuf; gpsimd bias-add -> pkb;
           premix (bd1 matmul); exp from psum (no max-sub: logits tiny);
           DVE sum+recip; normalizer folded into bd2 rows (gpsimd); postmix
           fused with transpose-back (lhsT=et), 4-chained psat groups;
           cast-evict -> atw; AV per head with b_w*colsum(v) added via
           per-partition bias at psav eviction; projection with b_proj as an
           extra contraction row; psum -> DRAM DMA.

b_l is softmax-invariant and dropped.  n = nb*25 + j (nb<10, j<25).
"""
import numpy as np
from contextlib import ExitStack

import concourse.bass as bass
import concourse.tile as tile
from concourse import bacc, mybir, library_config
from concourse.bass_utils import run_bass_kernel_spmd

F32 = mybir.dt.float32
F32R = mybir.dt.float32r
BF16 = mybir.dt.bfloat16
I32 = mybir.dt.int32
I16 = mybir.dt.int16
AX = mybir.AxisListType.X
EXP = mybir.ActivationFunctionType.Exp
ADD = mybir.AluOpType.add
MULT = mybir.AluOpType.mult

NCORES = 8
B, N, C, H, D = 64, 245, 768, 12, 64
BLOC = B // NCORES
E = 3 * C
NBKT = 1698
SCALE = D ** -0.5
NGRP = 10                   # nb groups; packed rows = NGRP*H = 120
NJ = 25                     # n = nb*NJ + j
PR = NGRP * H               # 120
NN = NGRP * NJ              # 250 (n padded; q cols 245..249 junk)
NP = 256                    # padded m stride in xT/qkT
GSZ = 4
CC = C // 128               # 6
MCS = [(0, 128), (128, 117)]
NI = NJ * N                 # 6125
NIW = 7664                  # balanced idx list length per q7 core


def _emit(ctx: ExitStack, tc, io):
    nc = tc.nc
    x_d, wqkv_d, wproj_d, bproj_d, wl_d, ww_d, bw_d, rpe_d, rel_d, idx_d, out_d = io

    const = ctx.enter_context(tc.tile_pool(name="const", bufs=1))
    ctx0 = ExitStack()
    tmp = ctx0.enter_context(tc.tile_pool(name="tmp", bufs=1))
    ps = ctx.enter_context(tc.tile_pool(name="ps", bufs=1, space="PSUM"))

    identB = const.tile([128, 128], BF16)
    from concourse.masks import make_identity
    make_identity(nc, identB[:])
    identF = const.tile([128, 128], F32)
    make_identity(nc, identF[:])

    # ---- small weights ----
    wl_sb = tmp.tile([12, 12], F32, tag="wsml")
    nc.sync.dma_start(out=wl_sb[:], in_=wl_d[:, :])
    ps12 = ps.tile([12, 12], F32, tag="a")
    nc.tensor.transpose(out=ps12[:], in_=wl_sb[:], identity=identF[:12, :12])
    wlT = tmp.tile([12, 12], BF16, tag="wsml2")
    nc.scalar.copy(out=wlT[:], in_=ps12[:])

    ww_sb = tmp.tile([12, 12], F32, tag="wsml")
    nc.sync.dma_start(out=ww_sb[:], in_=ww_d[:, :])
    ps12b = ps.tile([12, 12], F32, tag="b")
    nc.tensor.transpose(out=ps12b[:], in_=ww_sb[:], identity=identF[:12, :12])
    wwT = tmp.tile([12, 12], BF16, tag="wsml2b")
    nc.scalar.copy(out=wwT[:], in_=ps12b[:])

    bd1 = const.tile([128, PR], BF16)   # [(nb,h), (nb,g)] = wl[g,h]
    nc.vector.memset(bd1[:], 0.0)
    bd2 = const.tile([128, PR], BF16)   # [(nb,h), (nb,g)] = ww[g,h]
    nc.vector.memset(bd2[:], 0.0)
    for nb in range(NGRP):
        s = nb * H
        nc.gpsimd.dma_start(out=bd1[s:s + H, s:s + H], in_=wlT[:])
        nc.gpsimd.dma_start(out=bd2[s:s + H, s:s + H], in_=wwT[:])

    # b_w broadcast: bw_cols[p, h] = b_w[h] on 64 partitions
    bw_cols = const.tile([64, H], F32)
    for h in range(H):
        nc.gpsimd.dma_start(
            out=bw_cols[:, h:h + 1],
            in_=bw_d[h:h + 1].unsqueeze(0).to_broadcast([64, 1]))
    # b_proj as an extra (row-0-only) contraction step for the projection
    bproj_row = const.tile([1, C], F32)
    nc.sync.dma_start(out=bproj_row[:], in_=bproj_d[:].unsqueeze(0))
    e0_col = const.tile([128, 128], BF16)   # only row 0 is ones
    nc.vector.memset(e0_col[:], 0.0)
    nc.gpsimd.memset(e0_col[0:1, :], 1.0)
    bprow128 = const.tile([128, C], BF16)   # only row 0 is b_proj
    nc.vector.memset(bprow128[:], 0.0)
    nc.gpsimd.tensor_copy(out=bprow128[0:1, :], in_=bproj_row[:])
    ones_col = const.tile([128, 1], BF16)
    nc.vector.memset(ones_col[:], 1.0)

    # ---- big weights: load f32, PE-transpose (f32r), evict-cast to bf16 ----
    wqkvT = const.tile([128, CC, E], BF16)     # [c, cc, e]
    wprojT = const.tile([128, CC, C], BF16)
    for w_i, (w_d, wT, necs, scl) in enumerate((
            (wqkv_d, wqkvT, E // 128, True), (wproj_d, wprojT, CC, False))):
        for ec in range(necs):
            wt = tmp.tile([128, C], F32, tag="wload", bufs=2)
            nc.sync.dma_start(out=wt[:], in_=w_d[ec * 128:(ec + 1) * 128, :])
            wb = tmp.tile([128, C], BF16, tag="wcast", bufs=2)
            if scl and ec < 6:
                nc.vector.tensor_scalar_mul(wb[:], wt[:], SCALE)
            else:
                nc.scalar.copy(out=wb[:], in_=wt[:])
            for cc2 in range(3):   # 2 cc per psum tile
                psw = ps.tile([128, 2, 128], BF16, tag=("e", "f")[cc2 % 2],
                              name="psw")
                for k in range(2):
                    cc = cc2 * 2 + k
                    nc.tensor.matmul(
                        out=psw[:, k, :], lhsT=wb[:, cc * 128:(cc + 1) * 128],
                        rhs=identB[:, :], is_transpose=True,
                        start=(k == 0), stop=(k == 1))
                dst = wT[:, cc2 * 2:cc2 * 2 + 2, ec * 128:(ec + 1) * 128]
                if (ec + cc2) % 2 == 0:
                    nc.scalar.copy(out=dst, in_=psw[:])
                else:
                    nc.vector.tensor_copy(out=dst, in_=psw[:])

    # ---- RPE bias gather into packed rows [(nb,h), (j, m)] ----
    # Single balanced ap_gather: q7 core k's idx list covers nb=k plus a
    # 1/8 slice of nb 8-9 (idx lists host-wrapped as idx_w).
    rpe_sb = tmp.tile([12, NBKT], F32, tag="rpesb")
    nc.sync.dma_start(out=rpe_sb[:], in_=rpe_d[:, :])
    trep1 = tmp.tile([128, NBKT], F32, tag="trep1")
    nc.vector.memset(trep1[:], 0.0)
    for q in range(8):
        nc.gpsimd.dma_start(out=trep1[q * 16:q * 16 + 12, :], in_=rpe_sb[:, :])

    idx32 = tmp.tile([128, NIW // 16], I32, tag="idx32a")
    nc.sync.dma_start(out=idx32[:], in_=idx_d[:, :])
    idx16 = tmp.tile([128, NIW // 16], I16, tag="idx16a")
    nc.vector.tensor_copy(out=idx16[:], in_=idx32[:])

    nc.gpsimd.load_library(library_config.ap_gather)
    bias_full = tmp.tile([128, NIW], BF16, tag="bfull")
    c0 = 0
    for ncols in (120, 120, 120, 119):
        ni = ncols * 16
        bias_g = tmp.tile([128, 1920], F32, tag="bg", bufs=2)
        nc.gpsimd.ap_gather(
            out_ap=bias_g[:, 0:ni], in_ap=trep1[:].unsqueeze(2),
            idxs_ap=idx16[:, c0:c0 + ncols],
            channels=128, num_elems=NBKT, d=1, num_idxs=ni)
        if (c0 // 120) % 2 == 0:
            nc.vector.tensor_copy(out=bias_full[:, c0 * 16:c0 * 16 + ni],
                                  in_=bias_g[:, 0:ni])
        else:
            nc.scalar.copy(out=bias_full[:, c0 * 16:c0 * 16 + ni],
                           in_=bias_g[:, 0:ni])
        c0 += ncols
    nc.gpsimd.load_library(library_config.standard)

    bias_pk = const.tile([PR, NJ, N], BF16)
    for nb in range(8):
        nc.gpsimd.dma_start(
            out=bias_pk[nb * H:(nb + 1) * H, :, :],
            in_=bias_full[nb * 16:nb * 16 + H, 0:NI].rearrange(
                "p (j m) -> p j m", j=NJ))
    # nb 8-9 tails: core k's cols [NI, NI+1532) hold flat range
    # [k*1532, (k+1)*1532) of the nb>=8 region (12250 long)
    for k in range(8):
        f0 = k * 1532
        f1 = min(f0 + 1532, 2 * NI)
        while f0 < f1:
            nb = 8 + f0 // NI
            e = min(f1, (f0 // NI + 1) * NI)
            nc.gpsimd.dma_start(
                out=bias_pk[nb * H:(nb + 1) * H, :, :].rearrange(
                    "p j m -> p (j m)")[:, f0 - (nb - 8) * NI:e - (nb - 8) * NI],
                in_=bias_full[k * 16:k * 16 + H, NI + f0 - k * 1532:NI + e - k * 1532])
            f0 = e

    ctx0.close()

    # ---- streaming pools ----
    xb_p = ctx.enter_context(tc.tile_pool(name="xb", bufs=2))
    xT_p = ctx.enter_context(tc.tile_pool(name="xT", bufs=1))
    qkT_p = ctx.enter_context(tc.tile_pool(name="qkT", bufs=1))
    v_p = ctx.enter_context(tc.tile_pool(name="v", bufs=1))
    sq_p = ctx.enter_context(tc.tile_pool(name="sq", bufs=2))
    pk_p = ctx.enter_context(tc.tile_pool(name="pk", bufs=3))
    et_p = ctx.enter_context(tc.tile_pool(name="et", bufs=3))
    sm_p = ctx.enter_context(tc.tile_pool(name="sm", bufs=4))
    atw_p = ctx.enter_context(tc.tile_pool(name="atw", bufs=2))
    oT_p = ctx.enter_context(tc.tile_pool(name="oT", bufs=2))

    ev = [
        lambda out, in_: nc.scalar.copy(out=out, in_=in_),
        lambda out, in_: nc.vector.tensor_copy(out=out, in_=in_),
    ]

    for gg in range(BLOC // GSZ):
        # ---- phase A ----
        xT_all = xT_p.tile([128, CC, GSZ * NP], BF16, tag="xT")
        qkT = qkT_p.tile([128, 12, GSZ * NP], BF16, tag="qkT")
        v_all = v_p.tile([128, 2, GSZ, C], BF16, tag="v")
        for bs in range(GSZ):
            b = gg * GSZ + bs
            xb = xb_p.tile([128, 2, C], F32, tag="xb")
            for mc, (mo, msz) in enumerate(MCS):
                nc.sync.dma_start(out=xb[:msz, mc, :], in_=x_d[b, mo:mo + msz, :])
            nc.sync.dma_start(out=xb[117:128, 1, :], in_=x_d[b, 234:245, :])
            xbb = xb_p.tile([128, 2, C], BF16, tag="xbb")
            nc.scalar.copy(out=xbb[:, 0, :], in_=xb[:, 0, :])
            nc.scalar.copy(out=xbb[:, 1, :], in_=xb[:, 1, :])
            # x^T via bf16 PE transposes, 4 per psum tile, evicted to xT
            for c4 in range(3):   # (cc pair) x (mc pair) per tile
                psx = ps.tile([128, 2, 2, 128], BF16, tag=("e", "f")[c4 % 2],
                              name="psx")
                for k in range(2):
                    cc = c4 * 2 + k
                    for mc in range(2):
                        nc.tensor.matmul(
                            out=psx[:, k, mc, :],
                            lhsT=xbb[:, mc, cc * 128:(cc + 1) * 128],
                            rhs=identB[:, :], is_transpose=True,
                            start=(k == 0 and mc == 0), stop=(k == 1 and mc == 1))
                ev[(bs + c4) % 2](
                    xT_all[:, c4 * 2:c4 * 2 + 2, bs * NP:(bs + 1) * NP].rearrange(
                        "p c (q x) -> p c q x", q=2),
                    psx[:])
        # QKV (q scaled in weights): qkT [hd, n]
        for ec in range(12):
            pst = [ps.tile([128, NP], F32, tag=tg, name=f"psq{tg}")
                   for tg in ("a", "b", "c", "d")]
            for cc in range(CC):
                for bs in range(GSZ):
                    nc.tensor.matmul(
                        out=pst[bs][:, :],
                        lhsT=wqkvT[:, cc, ec * 128:(ec + 1) * 128],
                        rhs=xT_all[:, cc, bs * NP:(bs + 1) * NP],
                        start=(cc == 0), stop=(cc == CC - 1))
            for bs in range(GSZ):
                ev[(ec * GSZ + bs) % 2](
                    qkT[:, ec, bs * NP:(bs + 1) * NP], pst[bs][:, :])
        # V: [m, hd]
        for bs in range(GSZ):
            for mc, (mo, msz) in enumerate(MCS):
                psv0 = ps.tile([128, 384], F32, tag="c")
                psv1 = ps.tile([128, 384], F32, tag="d")
                psvs = (psv0, psv1)
                for cc in range(CC):
                    for vc in range(2):
                        nc.tensor.matmul(
                            out=psvs[vc][:msz, :],
                            lhsT=xT_all[:, cc, bs * NP + mo:bs * NP + mo + msz],
                            rhs=wqkvT[:, cc, 2 * C + vc * 384:2 * C + (vc + 1) * 384],
                            start=(cc == 0), stop=(cc == CC - 1))
                for vc in range(2):
                    ev[(bs * 2 + mc + vc) % 2](
                        v_all[:msz, mc, bs, vc * 384:(vc + 1) * 384],
                        psvs[vc][:msz, :])

        # ---- phase B ----
        for bs in range(GSZ):
            b = gg * GSZ + bs
            # QK^T per head -> evict into sqT [m, mc, nb, h, j] so that the
            # packed (nb,h) gather at fixed j is a single-stride AP (stride NJ)
            sqT = sq_p.tile([128, 2, NGRP, H, NJ], BF16, tag="sqT")
            for mc in range(2):
                for t in range(6):
                    for h2 in range(2):
                        h = 2 * t + h2
                        psqk = ps.tile([128, NN], F32,
                                       tag=("e", "f")[h % 2], name="psqk")
                        nc.tensor.matmul(
                            out=psqk[:, :],
                            lhsT=qkT[h2 * 64:(h2 + 1) * 64, 6 + t,
                                     bs * NP + mc * 128:bs * NP + (mc + 1) * 128],
                            rhs=qkT[h2 * 64:(h2 + 1) * 64, t, bs * NP:bs * NP + NN],
                            start=True, stop=True)
                        ev[(mc * 12 + h) % 2](
                            sqT[:, mc, :, h, :],
                            psqk[:, :].rearrange("p (nb j) -> p nb j", nb=NGRP))

            # packed attention middle, 2 j per step
            # atw layout [m, mc, g, n]: evictions de-interleave (nb,g);
            # column 250 is ones so AV also emits colsum(v) per head
            atw = atw_p.tile([128, 2, H, NN + 6], BF16, tag="atw")
            nc.vector.memset(atw[:, :, :, NN:NN + 1], 1.0)
            psat = [None, None]
            for jj in range(0, NJ, 2):
                js = min(2, NJ - jj)
                # pack-transpose: strided (nb,h)-gather input, 4-chained group
                pkT = ps.tile([128, 2, 2, 128], BF16, tag=("e", "f")[(jj // 2) % 2],
                              name="pkT")
                for i in range(js):
                    for mc in range(2):
                        src = sqT[:, mc].rearrange(
                            "p nb h j -> p (nb h) j")[:, :, jj + i]
                        nc.tensor.matmul(
                            out=pkT[0:PR, i, mc, :],
                            lhsT=src,
                            rhs=identB[:, :], is_transpose=True,
                            start=(i == 0 and mc == 0),
                            stop=(i == js - 1 and mc == 1))
                pkS = pk_p.tile([PR, 2, 2, 128], BF16, tag="pkS")
                ev[(jj // 2) % 2](pkS[:, 0:js, :, :], pkT[0:PR, 0:js, :, :])
                pkb = pk_p.tile([PR, 2, N], BF16, tag="pkb")
                nc.vector.tensor_tensor(
                    out=pkb[:, 0:js, :],
                    in0=pkS[:, 0:js, :, :].rearrange("p j m c -> p j (m c)")[:, :, 0:N],
                    in1=bias_pk[:, jj:jj + js, :], op=ADD)
                psm = ps.tile([PR, 2, N], F32, tag=("g", "h")[(jj // 2) % 2],
                              name="psm")
                nc.tensor.matmul(out=psm[:, 0:js, :], lhsT=bd1[0:PR, :],
                                 rhs=pkb[:, 0:js, :], start=True, stop=True)
                et = et_p.tile([PR, 2, N], BF16, tag="et")
                nc.scalar.activation(out=et[:, 0:js, :], in_=psm[:, 0:js, :], func=EXP)
                ssum = sm_p.tile([PR, 2], F32, tag="ss")
                nc.vector.tensor_reduce(out=ssum[:, 0:js], in_=et[:, 0:js, :],
                                        axis=AX, op=ADD)
                rec = sm_p.tile([PR, 2], F32, tag="rc")
                nc.vector.reciprocal(out=rec[:, 0:js], in_=ssum[:, 0:js])
                bd2j = sm_p.tile([PR, 2, PR], BF16, tag="b2")
                for i in range(js):
                    nc.vector.tensor_scalar_mul(bd2j[:, i, :], bd2[0:PR, :],
                                                rec[:, i:i + 1])
                for i in range(js):
                    j = jj + i
                    if j % 4 == 0:
                        psat[0] = ps.tile([128, 4, PR], F32, tag="a", name="psat0")
                        psat[1] = ps.tile([128, 4, PR], F32, tag="b", name="psat1")
                    for mc, (mo, msz) in enumerate(MCS):
                        nc.tensor.matmul(
                            out=psat[mc][:msz, j % 4, :],
                            lhsT=et[:, i, mo:mo + msz], rhs=bd2j[:, i, :],
                            start=(j % 4 == 0), stop=(j % 4 == 3 or j == NJ - 1))
                    if j % 4 == 3 or j == NJ - 1:
                        j0 = (j // 4) * 4
                        nj4 = j - j0 + 1
                        for mc, (mo, msz) in enumerate(MCS):
                            dst = atw[:msz, mc, :, 0:NN].rearrange(
                                "p g (nb j) -> p j nb g", nb=NGRP,
                                j=NJ)[:, j0:j0 + nj4]
                            ev[mc](dst,
                                   psat[mc][:msz, 0:nj4, :].rearrange(
                                       "p j (nb g) -> p j nb g", nb=NGRP))

            # AV per head; psav eviction adds b_w*colsum(v) per-partition
            outT = oT_p.tile([128, CC, N], BF16, tag="oT")
            bwcv = sm_p.tile([64, 12], F32, tag="bw")
            for t in range(6):
                psav0 = ps.tile([64, NN + 1], F32, tag="c")
                psav1 = ps.tile([64, NN + 1], F32, tag="d")
                psavs = (psav0, psav1)
                for h2 in range(2):
                    for mc, (mo, msz) in enumerate(MCS):
                        nc.tensor.matmul(
                            out=psavs[h2][:, :],
                            lhsT=v_all[:msz, mc, bs, (2 * t + h2) * 64:(2 * t + h2 + 1) * 64],
                            rhs=atw[:msz, mc, 2 * t + h2, 0:NN + 1],
                            start=(mc == 0), stop=(mc == 1))
                for h2 in range(2):
                    nc.vector.tensor_tensor(
                        out=bwcv[:, 2 * t + h2:2 * t + h2 + 1],
                        in0=psavs[h2][:, NN:NN + 1],
                        in1=bw_cols[:, 2 * t + h2:2 * t + h2 + 1], op=MULT)
                nc.scalar.add(out=outT[0:64, t, :], in_=psav0[:, 0:N],
                              add=bwcv[:, 2 * t:2 * t + 1])
                nc.scalar.add(out=outT[64:128, t, :], in_=psav1[:, 0:N],
                              add=bwcv[:, 2 * t + 1:2 * t + 2])

            # projection; b_proj enters as an extra contraction row
            for mc, (mo, msz) in enumerate(MCS):
                psy0 = ps.tile([128, 384], F32, tag="c")
                psy1 = ps.tile([128, 384], F32, tag="d")
                psyt = (psy0, psy1)
                for cc in range(CC):
                    for half in range(2):
                        nc.tensor.matmul(
                            out=psyt[half][:msz, :],
                            lhsT=outT[:, cc, mo:mo + msz],
                            rhs=wprojT[:, cc, half * 384:(half + 1) * 384],
                            start=(cc == 0), stop=False)
                for half in range(2):
                    nc.tensor.matmul(
                        out=psyt[half][:msz, :],
                        lhsT=e0_col[:, 0:msz],
                        rhs=bprow128[:, half * 384:(half + 1) * 384],
                        start=False, stop=True)
                y = sm_p.tile([128, C], F32, tag="y", bufs=2)
                for half in range(2):
                    ev[half](y[:msz, half * 384:(half + 1) * 384],
                             psyt[half][:msz, :])
                nc.sync.dma_start(out=out_d[b, mo:mo + msz, :], in_=y[:msz, :])


def _wrap_idx(rel_idx):
    NIWl = 7664
    rel_flat = np.asarray(rel_idx, np.int64).reshape(-1)
    lists = np.zeros((8, NIWl), np.int32)
    for k in range(8):
        lists[k, :6125] = rel_flat[k * 6125:(k + 1) * 6125]
        f0 = k * 1532
        f1 = min(f0 + 1532, 11025)   # rel data ends at n=245 (nb9 j>=20 pad)
        if f1 > f0:
            lists[k, 6125:6125 + f1 - f0] = rel_flat[49000 + f0:49000 + f1]
    w = lists.reshape(8, NIWl // 16, 16).transpose(0, 2, 1).reshape(128, NIWl // 16)
    return np.ascontiguousarray(w.astype(np.int32))


_CACHE = {}


def _build():
    if "nc" in _CACHE:
        return _CACHE["nc"]
    nc = bacc.Bacc("TRN2", target_bir_lowering=False, debug=False, num_devices=NCORES)
    io = (
        nc.dram_tensor("x", [BLOC, N, C], F32, kind="ExternalInput").ap(),
        nc.dram_tensor("w_qkv", [E, C], F32, kind="ExternalInput").ap(),
        nc.dram_tensor("w_proj", [C, C], F32, kind="ExternalInput").ap(),
        nc.dram_tensor("b_proj", [C], F32, kind="ExternalInput").ap(),
        nc.dram_tensor("w_l", [H, H], F32, kind="ExternalInput").ap(),
        nc.dram_tensor("w_w", [H, H], F32, kind="ExternalInput").ap(),
        nc.dram_tensor("b_w", [H], F32, kind="ExternalInput").ap(),
        nc.dram_tensor("rpe_table", [H, NBKT], F32, kind="ExternalInput").ap(),
        nc.dram_tensor("rel_idx", [N, N], I32, kind="ExternalInput").ap(),
        nc.dram_tensor("idx_w", [128, 7664 // 16], I32, kind="ExternalInput").ap(),
        nc.dram_tensor("out", [BLOC, N, C], F32, kind="ExternalOutput").ap(),
    )
    with tile.TileContext(nc) as tc, ExitStack() as ctx:
        _emit(ctx, tc, io)
    nc.compile()
    _CACHE["nc"] = nc
    return nc


def kernel(x, w_qkv, w_proj, b_proj, w_l, b_l, w_w, b_w, rpe_table, rel_idx,
           _trace=False):
    nc = _build()
    shared = {
        "w_qkv": np.ascontiguousarray(w_qkv, np.float32),
        "w_proj": np.ascontiguousarray(w_proj, np.float32),
        "b_proj": np.ascontiguousarray(b_proj, np.float32),
        "w_l": np.ascontiguousarray(w_l, np.float32),
        "w_w": np.ascontiguousarray(w_w, np.float32),
        "b_w": np.ascontiguousarray(b_w, np.float32),
        "rpe_table": np.ascontiguousarray(rpe_table, np.float32),
        "rel_idx": np.ascontiguousarray(rel_idx, np.int32),
        "idx_w": _wrap_idx(rel_idx),
    }
    x = np.ascontiguousarray(x, np.float32)
    in_maps = [dict(shared, x=x[i * BLOC:(i + 1) * BLOC]) for i in range(NCORES)]
    res = run_bass_kernel_spmd(nc, in_maps, core_ids=list(range(NCORES)),
                               trace=_trace)
    out = np.concatenate([res.results[i]["out"] for i in range(NCORES)], axis=0)
    if _trace:
        kernel.last_result = res
    return out

